# revision 6
# baseline (speedup 1.0000x reference)
"""Trainium2 Bass kernel for the DivTree per-agent MoE MLP problem.

Math (see reference): for each agent c of 64,
  x2[:, c, :]      = relu(x[:, c, :] @ W1[r[c]][:H] + bias_A[c]) @ W2[r[c]] + b2[r[c]]
  confact[:, c, :] = relu(x[:, c, :] @ W1[r'[c]][:H] + bias_B[c]) @ W2[r'[c]] + b2[r'[c]]
where r = routing, r'[c] = routing[pinv[c]] with pinv the inverse of perm_index,
and the one-hot agent-id concat folds into the bias:
  bias_A[c] = b1[r[c]] + W1[r[c]][H + c],  bias_B[c] = b1[r'[c]] + W1[r'[c]][H + pinv[c]].
(The counterfactual output confact[:, perm[a], :] = net_a(x[:, perm[a], :]) — both
passes read the SAME input column per output column, only weights/biases differ.)

Sharding: expert/agent-parallel — 8 agents per core, weights host-gathered per
agent so each core runs a dense per-agent 512->512->32 MLP over the full batch.
All tensors are fed pre-swizzled from the host, x already h-major (transposed),
so the device does only matmuls + fused bias/relu epilogues.

Batch is processed in 2 units of 1024 so each PSUM tile spans 2 banks and the
bias+relu epilogue runs as one [128, 1024] op — amortizing the ~550ns
PSUM-access fixed cost per VectorE/ScalarE op (the throughput limiter).

Device layout per core:
  XT  [128, 8, 2, 4, 1024] x^T: XT[hi, a, u, kc, f] = x[u*1024+f, ag0+a, kc*128+hi]
  W1S [128, 8, 2, 4, 512]  lhsT layer-1 (k-partition, k-chunk, m)
  W2S [128, 8, 2, 4, 32]   lhsT layer-2
  B1S [128, 8, 2, 4]       per-partition bias for h' tiles
  B2S [32, 8, 2]           per-partition bias for output
  OUT [2, 8, 32, 2048]     (set, agent, o, b) transposed output

Matmuls run in bfloat16 (1 cycle/row on the PE — hardware fp32r streams at
~1/4 rate, unlike the CoreSim model) with fp32 PSUM accumulation; biases are
applied in fp32 epilogues, so end-to-end rel err is ~4e-3 (budget 2e-2).
bf16 also halves the x/W DMA traffic. Epilogues alternate between VectorE
and ScalarE to balance the two engines.
"""

import ml_dtypes
import numpy as np

import concourse.bass as bass
import concourse.mybir as mybir
from concourse import bacc
from concourse.tile import TileContext
from concourse.bass_utils import run_bass_kernel_spmd

F32 = mybir.dt.float32
BF16 = mybir.dt.bfloat16
NPBF16 = ml_dtypes.bfloat16
AF = mybir.ActivationFunctionType
ALU = mybir.AluOpType

B, A, H, O = 2048, 64, 512, 32
NCORES = 8
AL = A // NCORES          # agents per core
NU = 2                    # batch units of 1024
U = B // NU               # 1024
KC = H // 128             # 4
MT = H // 128             # 4 output tiles of layer 1

_CACHED = {}


def _build_nc(repeat=1):
    nc = bacc.Bacc("TRN2", target_bir_lowering=False, debug=False,
                   num_devices=NCORES)
    xs = nc.dram_tensor("xs", [128, AL, NU, KC, U], BF16,
                        kind="ExternalInput")
    w1 = nc.dram_tensor("w1", [128, AL, 2, KC, H], BF16, kind="ExternalInput")
    w2 = nc.dram_tensor("w2", [128, AL, 2, KC, 128], BF16, kind="ExternalInput")
    b1 = nc.dram_tensor("b1", [128, AL, 2, MT], F32, kind="ExternalInput")
    b2 = nc.dram_tensor("b2", [32, AL, 2], F32, kind="ExternalInput")
    out = nc.dram_tensor("out", [2, AL, O, B], F32, kind="ExternalOutput")

    with TileContext(nc) as tc:
        with (
            tc.tile_pool(name="weights", bufs=1) as wpool,
            tc.tile_pool(name="w1s", bufs=4) as w1pool,
            tc.tile_pool(name="xT", bufs=2) as xtpool,
            tc.tile_pool(name="hT", bufs=4) as hpool,
            tc.tile_pool(name="ob", bufs=3) as opool,
            tc.tile_pool(name="ps1", bufs=3, space="PSUM") as ps1_pool,
            tc.tile_pool(name="ps2", bufs=2, space="PSUM") as ps2_pool,
        ):
            w2t = wpool.tile([128, AL, 2, KC, 128], BF16)
            nc.sync.dma_start(w2t[:], w2[:])
            b1t = wpool.tile([128, AL, 2, MT], F32)
            nc.sync.dma_start(b1t[:], b1[:])
            b2t = wpool.tile([32, AL, 2], F32)
            nc.sync.dma_start(b2t[:], b2[:])

            def layer2_set(hT, a, s, u, ob):
                for half in range(2):
                    ps2 = ps2_pool.tile([128, 512], F32)
                    for kc in range(KC):
                        nc.tensor.matmul(
                            ps2[:],
                            w2t[:, a, s, kc, :],
                            hT[:, kc, half * 512:(half + 1) * 512],
                            start=(kc == 0), stop=(kc == KC - 1),
                        )
                    oslc = ob[:, half * 512:(half + 1) * 512]
                    if (s + half) % 2 == 0:
                        nc.scalar.activation(
                            oslc, ps2[:32, :], AF.Identity,
                            bias=b2t[:, a, s:s + 1],
                        )
                    else:
                        nc.vector.tensor_scalar_add(
                            oslc, ps2[:32, :], b2t[:, a, s:s + 1])
                nc.sync.dma_start(out[s, a, :, u * U:(u + 1) * U], ob[:])

            def layer1_set(xT, w1t, a, s, u):
                hT = hpool.tile([128, KC, U], BF16)
                for mt in range(MT):
                    ps1 = ps1_pool.tile([128, U], F32)
                    for kc in range(KC):
                        lhsT = w1t[:, kc, mt * 128:(mt + 1) * 128]
                        for half in range(2):
                            nc.tensor.matmul(
                                ps1[:, half * 512:(half + 1) * 512],
                                lhsT,
                                xT[:, kc, half * 512:(half + 1) * 512],
                                start=(kc == 0), stop=(kc == KC - 1),
                            )
                    if mt % 2 == 0:
                        nc.vector.tensor_scalar(
                            hT[:, mt, :], ps1[:],
                            b1t[:, a, s, mt:mt + 1], 0.0,
                            ALU.add, ALU.max,
                        )
                    else:
                        nc.scalar.activation(
                            hT[:, mt, :], ps1[:], AF.Relu,
                            bias=b1t[:, a, s, mt:mt + 1],
                        )
                return hT

            def body():
                # half-unit software skew: each layer-2 set-job runs two
                # L1-sets after its inputs were produced, so the PE always
                # has independent matmuls covering the epilogue latency
                pending = []
                for a in range(AL):
                    w1ts = []
                    for s in range(2):
                        w1t = w1pool.tile([128, KC, H], BF16)
                        w1ts.append(w1t)
                        nc.sync.dma_start(w1t[:], w1[:, a, s, :, :])
                    for u in range(NU):
                        xT = xtpool.tile([128, KC, U], BF16)
                        nc.sync.dma_start(xT[:], xs[:, a, u, :, :])
                        for s in range(2):
                            hT = layer1_set(xT, w1ts[s], a, s, u)
                            ob = opool.tile([32, U], F32)
                            pending.append((hT, a, s, u, ob))
                            if len(pending) > 2:
                                layer2_set(*pending.pop(0))
                for job in pending:
                    layer2_set(*job)

            for _ in range(repeat):
                body()
    nc.compile()
    return nc


def _host_prep(x_in, W1, b1, W2, b2, routing, perm_index):
    routing = np.asarray(routing).astype(np.int64)
    perm = np.asarray(perm_index).astype(np.int64)
    pinv = np.empty(A, dtype=np.int64)
    pinv[perm] = np.arange(A)

    eA = routing
    eB = routing[pinv]
    idA = np.arange(A)
    idB = pinv

    # [A, 2, H, H] layer-1 weights (one-hot rows folded into bias)
    W1s = np.stack([W1[eA, :H, :], W1[eB, :H, :]], axis=1)
    b1s = np.stack(
        [b1[eA] + W1[eA, H + idA, :], b1[eB] + W1[eB, H + idB, :]], axis=1)
    W2s = np.stack([W2[eA], W2[eB]], axis=1)      # [A, 2, H, O]
    b2s = np.stack([b2[eA], b2[eB]], axis=1)      # [A, 2, O]

    # x^T swizzle: [hi, ag, u, kc, f]
    xT_all = x_in.reshape(NU, U, A, KC, 128).transpose(4, 2, 0, 3, 1)

    in_maps = []
    for c in range(NCORES):
        ag = slice(c * AL, (c + 1) * AL)
        in_maps.append({
            "xs": np.ascontiguousarray(xT_all[:, ag]).astype(NPBF16),
            "w1": np.ascontiguousarray(
                W1s[ag].reshape(AL, 2, KC, 128, H).transpose(3, 0, 1, 2, 4)
            ).astype(NPBF16),
            "w2": np.ascontiguousarray(np.tile(
                W2s[ag].reshape(AL, 2, KC, 128, O), (1, 1, 1, 1, 4)
            ).transpose(3, 0, 1, 2, 4)).astype(NPBF16),
            "b1": np.ascontiguousarray(
                b1s[ag].reshape(AL, 2, MT, 128).transpose(3, 0, 1, 2)),
            "b2": np.ascontiguousarray(b2s[ag].transpose(2, 0, 1)),
        })
    return in_maps


def kernel(x_in, W1, b1, W2, b2, routing, perm_index):
    x_in = np.asarray(x_in, dtype=np.float32)
    W1 = np.asarray(W1, dtype=np.float32)
    b1 = np.asarray(b1, dtype=np.float32)
    W2 = np.asarray(W2, dtype=np.float32)
    b2 = np.asarray(b2, dtype=np.float32)

    if "nc" not in _CACHED:
        _CACHED["nc"] = _build_nc()
    nc = _CACHED["nc"]

    in_maps = _host_prep(x_in, W1, b1, W2, b2, routing, perm_index)
    res = run_bass_kernel_spmd(nc, in_maps, list(range(NCORES)))

    x2 = np.empty((B, A, O), dtype=np.float32)
    confact = np.empty((B, A, O), dtype=np.float32)
    for c in range(NCORES):
        o = res.results[c]["out"]                 # [2, AL, O, B]
        x2[:, c * AL:(c + 1) * AL, :] = o[0].transpose(2, 0, 1)
        confact[:, c * AL:(c + 1) * AL, :] = o[1].transpose(2, 0, 1)
    return x2, confact



# revision 8
# speedup vs baseline: 1.1212x; 1.1212x over previous
"""Trainium2 Bass kernel for the DivTree per-agent MoE MLP problem.

Math (see reference): for each agent c of 64,
  x2[:, c, :]      = relu(x[:, c, :] @ W1[r[c]][:H] + bias_A[c]) @ W2[r[c]] + b2[r[c]]
  confact[:, c, :] = relu(x[:, c, :] @ W1[r'[c]][:H] + bias_B[c]) @ W2[r'[c]] + b2[r'[c]]
where r = routing, r'[c] = routing[pinv[c]] with pinv the inverse of perm_index,
and the one-hot agent-id concat folds into the bias:
  bias_A[c] = b1[r[c]] + W1[r[c]][H + c],  bias_B[c] = b1[r'[c]] + W1[r'[c]][H + pinv[c]].

Work dedup: when r'[c] == r[c] the confact column equals the x2 column, so
only ONE pass is computed for that column (the copy happens host-side).
The remaining distinct (column, weight-set) passes are split at unit
granularity (1024 of the 2048 batch) into unit-jobs and distributed evenly
over the 8 cores: with the staged routing/perm, 116 distinct passes x 2
units = 232 unit-jobs = 29 per core exactly (vs 32 for the dense layout).

Device program (SPMD, uniform across cores) is organized in column-slots:
a slot owns one x^T unit tile; pair slots run 2 unit-jobs (both weight
sets of one column) off that tile, single slots run 1. Slot/job counts
are computed from the actual routing/perm at kernel() time and the
compiled program is cached per shape.

Per-core device layout:
  xs  [128, NS, KC, U]   x^T slot tiles: xs[hi, t, kc, f] = x[..., kc*128+hi]
  w1  [128, NJ, KC, H]   layer-1 lhsT per unit-job (k-part, k-chunk, m)
  w2  [128, NJ, KC, 128] layer-2 lhsT (O=32 replicated x4 in m)
  b1  [128, NJ, MT]      per-partition layer-1 bias (one-hot folded in)
  b2  [32, NJ]           per-partition layer-2 bias
  out [NJ, O, U]         transposed per-unit-job output

Matmuls run in float32r (full-rate fp32 PE mode on HW; measured faster
than bf16 end-to-end on this workload). PSUM accumulates fp32; epilogues
(bias+relu / bias) alternate between VectorE and ScalarE.
"""

import ml_dtypes
import numpy as np

import concourse.bass as bass
import concourse.mybir as mybir
from concourse import bacc
from concourse.tile import TileContext
from concourse.bass_utils import run_bass_kernel_spmd

F32 = mybir.dt.float32
F32R = mybir.dt.float32r
BF16 = mybir.dt.bfloat16
NPBF16 = ml_dtypes.bfloat16
AF = mybir.ActivationFunctionType
ALU = mybir.AluOpType

B, A, H, O = 2048, 64, 512, 32
NCORES = 8
NU = 2                    # batch units per full pass
U = B // NU               # 1024
KC = H // 128             # 4 k-chunks
MT = H // 128             # 4 layer-1 output tiles

DT_IN = F32R              # matmul input dtype (F32R or BF16)
NP_IN = np.float32 if DT_IN == F32R else NPBF16

_CACHED = {}


def _build_nc(repeat=1, n2=13, n1=3):
    """n2 pair slots (2 unit-jobs each) + n1 single slots (1 each)."""
    ns = n2 + n1
    nj = 2 * n2 + n1
    nc = bacc.Bacc("TRN2", target_bir_lowering=False, debug=False,
                   num_devices=NCORES)
    xs = nc.dram_tensor("xs", [128, ns, KC, U], DT_IN, kind="ExternalInput")
    w1 = nc.dram_tensor("w1", [128, nj, KC, H], DT_IN, kind="ExternalInput")
    w2 = nc.dram_tensor("w2", [128, nj, KC, 128], DT_IN,
                        kind="ExternalInput")
    b1 = nc.dram_tensor("b1", [128, nj, MT], F32, kind="ExternalInput")
    b2 = nc.dram_tensor("b2", [32, nj], F32, kind="ExternalInput")
    out = nc.dram_tensor("out", [nj, O, U], F32, kind="ExternalOutput")

    with TileContext(nc) as tc:
        with (
            tc.tile_pool(name="weights", bufs=1) as wpool,
            tc.tile_pool(name="w1s", bufs=4) as w1pool,
            tc.tile_pool(name="xT", bufs=3) as xtpool,
            tc.tile_pool(name="hT", bufs=3) as hpool,
            tc.tile_pool(name="ob", bufs=3) as opool,
            tc.tile_pool(name="ps1", bufs=3, space="PSUM") as ps1_pool,
            tc.tile_pool(name="ps2", bufs=2, space="PSUM") as ps2_pool,
        ):
            w2t = wpool.tile([128, nj, KC, 128], DT_IN)
            nc.sync.dma_start(w2t[:], w2[:])
            b1t = wpool.tile([128, nj, MT], F32)
            nc.sync.dma_start(b1t[:], b1[:])
            b2t = wpool.tile([32, nj], F32)
            nc.sync.dma_start(b2t[:], b2[:])

            # epilogue ops alternate DVE/Act via a running counter
            ecnt = [0]

            def epi1(dst, src, bias_ap):
                if ecnt[0] % 2 == 0:
                    nc.vector.tensor_scalar(dst, src, bias_ap, 0.0,
                                            ALU.add, ALU.max)
                else:
                    nc.scalar.activation(dst, src, AF.Relu, bias=bias_ap)
                ecnt[0] += 1

            def epi2(dst, src, bias_ap):
                if ecnt[0] % 2 == 0:
                    nc.scalar.activation(dst, src, AF.Identity, bias=bias_ap)
                else:
                    nc.vector.tensor_scalar_add(dst, src, bias_ap)
                ecnt[0] += 1

            def layer1(xT, w1t, j):
                hT = hpool.tile([128, KC, U], DT_IN)
                for mt in range(MT):
                    ps1 = ps1_pool.tile([128, U], F32)
                    for kc in range(KC):
                        lhsT = w1t[:, kc, mt * 128:(mt + 1) * 128]
                        for half in range(2):
                            nc.tensor.matmul(
                                ps1[:, half * 512:(half + 1) * 512],
                                lhsT,
                                xT[:, kc, half * 512:(half + 1) * 512],
                                start=(kc == 0), stop=(kc == KC - 1),
                            )
                    epi1(hT[:, mt, :], ps1[:], b1t[:, j, mt:mt + 1])
                return hT

            def layer2(hT, j, ob):
                for half in range(2):
                    ps2 = ps2_pool.tile([128, 512], F32)
                    for kc in range(KC):
                        nc.tensor.matmul(
                            ps2[:],
                            w2t[:, j, kc, :],
                            hT[:, kc, half * 512:(half + 1) * 512],
                            start=(kc == 0), stop=(kc == KC - 1),
                        )
                    epi2(ob[:, half * 512:(half + 1) * 512], ps2[:32, :],
                         b2t[:, j:j + 1])
                nc.sync.dma_start(out[j, :, :], ob[:])

            def body():
                # 1-job software skew: layer2(t) issues after layer1(t+1),
                # so the PE always has independent matmuls covering the
                # PSUM-epilogue latency of hT(t).
                pending = []
                for t in range(ns):
                    xT = xtpool.tile([128, KC, U], DT_IN)
                    nc.sync.dma_start(xT[:], xs[:, t, :, :])
                    jobs = [2 * t, 2 * t + 1] if t < n2 else [n2 + t]
                    for j in jobs:
                        w1t = w1pool.tile([128, KC, H], DT_IN)
                        nc.sync.dma_start(w1t[:], w1[:, j, :, :])
                        hT = layer1(xT, w1t, j)
                        ob = opool.tile([32, U], F32)
                        pending.append((hT, j, ob))
                        if len(pending) > 1:
                            layer2(*pending.pop(0))
                for job in pending:
                    layer2(*job)

            for _ in range(repeat):
                body()
    nc.compile()
    return nc


def _plan(routing, perm_index):
    """Job/slot plan from the actual routing+perm.

    Returns (n2, n1, slot_map) with slot_map[core] = list of slots; each
    slot is a list of 1-2 job descriptors (col, expert, onehot_idx, set, u)
    or None entries for padding jobs.
    """
    routing = np.asarray(routing).astype(np.int64)
    perm = np.asarray(perm_index).astype(np.int64)
    pinv = np.empty(A, dtype=np.int64)
    pinv[perm] = np.arange(A)
    eA, eB = routing, routing[pinv]

    pair_cols = [c for c in range(A) if eA[c] != eB[c]]
    sing_cols = [c for c in range(A) if eA[c] == eB[c]]

    # unit-granular slots: (col, u) pairs / singles
    pair_slots = [(c, u) for c in pair_cols for u in range(NU)]
    sing_slots = [(c, u) for c in sing_cols for u in range(NU)]

    n2 = -(-len(pair_slots) // NCORES)
    # pad pair slots to n2*NCORES by degrading... simpler: move overflow
    # singles into pair slots as duplicated jobs. Pad with copies.
    while len(pair_slots) % NCORES:
        pair_slots.append(pair_slots[0])          # pad pair slot (dup col)
    n2 = len(pair_slots) // NCORES
    n1 = -(-len(sing_slots) // NCORES)
    while len(sing_slots) % NCORES or len(sing_slots) // NCORES < n1:
        sing_slots.append(sing_slots[0])          # pad single slot
    n1 = len(sing_slots) // NCORES

    def jobdesc(c, s, u):
        if s == 0:
            return (c, int(eA[c]), c, 0, u)
        return (c, int(eB[c]), int(pinv[c]), 1, u)

    slot_map = []
    for core in range(NCORES):
        slots = []
        for t in range(n2):
            c, u = pair_slots[core * n2 + t]
            slots.append([jobdesc(c, 0, u), jobdesc(c, 1, u)])
        for t in range(n1):
            c, u = sing_slots[core * n1 + t]
            slots.append([jobdesc(c, 0, u)])
        slot_map.append(slots)
    return n2, n1, slot_map


def _host_prep(x_in, W1, b1, W2, b2, routing, perm_index):
    n2, n1, slot_map = _plan(routing, perm_index)
    ns, nj = n2 + n1, 2 * n2 + n1

    x_in = np.asarray(x_in, dtype=np.float32)
    W1 = np.asarray(W1, dtype=np.float32)
    b1 = np.asarray(b1, dtype=np.float32)
    W2 = np.asarray(W2, dtype=np.float32)
    b2 = np.asarray(b2, dtype=np.float32)

    # x^T: [hi, c, u, kc, f]
    xT_all = np.ascontiguousarray(
        x_in.reshape(NU, U, A, KC, 128).transpose(4, 2, 0, 3, 1))
    # per-expert weight blocks, pre-swizzled once (<=64 experts but only
    # the used ones get touched)
    used = sorted({jd[1] for slots in slot_map for sl in slots for jd in sl})
    w1e = {e: np.ascontiguousarray(
        W1[e, :H, :].reshape(KC, 128, H).transpose(1, 0, 2)).astype(NP_IN)
        for e in used}                       # [128, KC, H]
    w2e = {e: np.ascontiguousarray(np.tile(
        W2[e].reshape(KC, 128, O), (1, 1, 4)).transpose(1, 0, 2)
        ).astype(NP_IN)                      # [128, KC, 128]
        for e in used}

    in_maps = []
    for core in range(NCORES):
        slots = slot_map[core]
        xs_c = np.empty((128, ns, KC, U), dtype=NP_IN)
        w1_c = np.empty((128, nj, KC, H), dtype=NP_IN)
        w2_c = np.empty((128, nj, KC, 128), dtype=NP_IN)
        b1_c = np.empty((128, nj, MT), dtype=np.float32)
        b2_c = np.empty((32, nj), dtype=np.float32)
        for t, sl in enumerate(slots):
            c, u = sl[0][0], sl[0][4]
            xs_c[:, t] = xT_all[:, c, u]
            for k, jd in enumerate(sl):
                j = 2 * t + k if t < n2 else n2 + t
                cc, e, oh, _s, _u = jd
                w1_c[:, j] = w1e[e]
                w2_c[:, j] = w2e[e]
                b1_c[:, j] = (b1[e] + W1[e, H + oh, :]).reshape(MT, 128).T
                b2_c[:, j] = b2[e]
        in_maps.append({"xs": xs_c, "w1": w1_c, "w2": w2_c,
                        "b1": b1_c, "b2": b2_c})
    return in_maps


def _plan_kwargs(in_maps):
    ns = in_maps[0]["xs"].shape[1]
    nj = in_maps[0]["w1"].shape[1]
    return {"n2": nj - ns, "n1": 2 * ns - nj}


def kernel(x_in, W1, b1, W2, b2, routing, perm_index):
    n2, n1, slot_map = _plan(routing, perm_index)
    key = (n2, n1)
    if key not in _CACHED:
        _CACHED[key] = _build_nc(n2=n2, n1=n1)
    nc = _CACHED[key]

    in_maps = _host_prep(x_in, W1, b1, W2, b2, routing, perm_index)
    res = run_bass_kernel_spmd(nc, in_maps, list(range(NCORES)))

    x2 = np.empty((B, A, O), dtype=np.float32)
    confact = np.empty((B, A, O), dtype=np.float32)
    seen = set()
    for core in range(NCORES):
        o = res.results[core]["out"]              # [nj, O, U]
        for t, sl in enumerate(slot_map[core]):
            for k, jd in enumerate(sl):
                j = 2 * t + k if t < n2 else n2 + t
                c, _e, _oh, s, u = jd
                if (c, s, u) in seen:
                    continue                      # padding duplicate
                seen.add((c, s, u))
                dst = x2 if s == 0 else confact
                dst[u * U:(u + 1) * U, c, :] = o[j].T
    # columns whose counterfactual expert equals the factual one
    routing = np.asarray(routing).astype(np.int64)
    perm = np.asarray(perm_index).astype(np.int64)
    pinv = np.empty(A, dtype=np.int64)
    pinv[perm] = np.arange(A)
    same = routing == routing[pinv]
    confact[:, same, :] = x2[:, same, :]
    return x2, confact
